# revision 8
# baseline (speedup 1.0000x reference)
"""AttentionTSSA Trainium2 kernel — full-IO contract.

kernel(**inputs) takes the FULL inputs (x [8,512,128,128], qkv_w, temp,
out_w, out_b), shards data-parallel over batch across the 8 NeuronCores
(batch i -> core i), runs a Bass/Tile kernel per core, and returns the
full [8,512,128,128] float32 output.

Per-core computation, t-major layout: all big operands are
[128 part, 32 tiles * (4 chunks * 512 tokens)] so every engine op and
DMA is a contiguous [128, 512..2048] slice.

  P1 (PE-bound): w = qkv_w @ x (fp16 matmuls); DVE copies w PSUM->SBUF;
     ACT/DVE square w with fused norm2 accumulation, sq spilled to DRAM.
  P2: logits[h,n] = sum_d w^2 * invnorm2 via masked-lhsT matmuls into
     [8,1024] strips; softmax over heads via exp -> ones-matmul ->
     Ln -> exp(-lnS) (division-free); Pi broadcast head->64-partition
     blocks via SBUF->SBUF DMA (stride-0 partition reads); dots
     accumulated by 4 all-SBUF f16 STTs per tile.
  P3 (PE-bound): o = w * Pi_b (one grouped DVE mult per tile);
     y = (out_w * -attn) @ o + b (fp16 matmuls, attn pre-folded into
     the weights with 4 per-partition scalar ops); y written as f16.

After compile, redundant ACT-table loads are collapsed into a single
natural_log_exp_and_others load (contains exp/ln/square/copy/identity).
"""

import sys

sys.path.insert(0, "/opt/trn_rl_repo")

from contextlib import ExitStack

import numpy as np

import concourse.bass as bass
import concourse.tile as tile
from concourse import bacc, mybir
from concourse.bass_utils import run_bass_kernel_spmd
from concourse.hw_specs import get_activation_tables

F32 = mybir.dt.float32
F16 = mybir.dt.float16   # value paths: x, w, sq, Pi, o, weights, y
AF = mybir.ActivationFunctionType
ALU = mybir.AluOpType

B = 8            # batch == number of cores
C = 512          # channels
H_IMG, W_IMG = 128, 128
N = H_IMG * W_IMG
HEADS = 8
HD = 64          # head dim
NT = 512         # tokens per chunk
KD = 4           # 128-partition chunks of the channel dim
P = 128
TW = KD * NT     # tile width in the t-major layout (2048)
G = 2            # tiles per softmax group (strip FD = G*NT = 1024)
LM_SCALE = 256.0  # keeps invnorm2 out of fp16-subnormal range in lmat

_NC_CACHE = {}


def _dedupe_act_table_loads(nc):
    """Collapse all InstLoadActFuncSet into one load of the set that
    contains every function this kernel uses (exp, ln, square, copy,
    identity). The kernel CFG is a single linear block per engine, so a
    single leading load is sufficient."""
    tables = list(get_activation_tables(nc.m.arch).keys())
    want = {AF.Exp, AF.Ln, AF.Square, AF.Copy, AF.Identity}
    sets = get_activation_tables(nc.m.arch)
    target = None
    for idx, name in enumerate(tables):
        if want <= sets[name]:
            target = idx
            break
    if target is None:
        return
    first = True
    for blk in nc.main_func.blocks:
        keep = []
        for inst in blk.instructions:
            if isinstance(inst, mybir.InstLoadActFuncSet):
                si = inst.sync_info
                has_sync = si is not None and (
                    len(si.on_wait) > 0 or len(si.on_update) > 0)
                if first or has_sync:
                    inst.act_func_set_id = target
                    first = False
                    keep.append(inst)
            else:
                keep.append(inst)
        blk.instructions[:] = keep


def _build_nc(n_tokens=N, n_cores=B):
    NTILES = n_tokens // NT          # 32
    NG = NTILES // G                 # softmax groups
    GW = G * NT                      # strip width (1024)
    TOT = NTILES * TW                # 65536 columns in t-major layout
    nc = bacc.Bacc("TRN2", target_bir_lowering=False, debug=False,
                   num_devices=n_cores)

    xb = nc.dram_tensor("xb", [P, TOT], F16, kind="ExternalInput").ap()
    qkvwT = nc.dram_tensor("qkvwT", [C, C], F16, kind="ExternalInput").ap()
    outwT = nc.dram_tensor("outwT", [C, C], F16, kind="ExternalInput").ap()
    lgmask = nc.dram_tensor("lgmask", [P, KD * HEADS], F16,
                            kind="ExternalInput").ap()
    ones8 = nc.dram_tensor("ones8", [HEADS, HEADS], F16,
                           kind="ExternalInput").ap()
    maskp = nc.dram_tensor("maskp", [HEADS, P], F16,
                           kind="ExternalInput").ap()
    ind2 = nc.dram_tensor("ind2", [HEADS, KD], F16,
                          kind="ExternalInput").ap()
    temp_s = nc.dram_tensor("temp_s", [HEADS, 1], F32,
                            kind="ExternalInput").ap()
    outb = nc.dram_tensor("outb", [P, KD], F32, kind="ExternalInput").ap()
    y = nc.dram_tensor("y", [P, TOT], F16, kind="ExternalOutput").ap()
    sq_dram = nc.dram_tensor("sq_scratch", [P, TOT], F16).ap()

    with tile.TileContext(nc) as tc, ExitStack() as top:
        const = top.enter_context(tc.tile_pool(name="const", bufs=1))
        persist = top.enter_context(tc.tile_pool(name="persist", bufs=1))

        # --- constants into SBUF -------------------------------------------
        qkvwT_sb = [const.tile([P, C], F16, name=f"qkvwT{k}") for k in range(KD)]
        outwT_sb = [const.tile([P, C], F16, name=f"outwT{k}") for k in range(KD)]
        for k in range(KD):
            nc.sync.dma_start(qkvwT_sb[k][:], qkvwT[k * P:(k + 1) * P, :])
            nc.sync.dma_start(outwT_sb[k][:], outwT[k * P:(k + 1) * P, :])
        lgmask_sb = const.tile([P, KD * HEADS], F16, name="lgmask")
        nc.sync.dma_start(lgmask_sb[:], lgmask)
        ones8_sb = const.tile([HEADS, HEADS], F16, name="ones8")
        nc.sync.dma_start(ones8_sb[:], ones8)
        maskp_sb = const.tile([HEADS, P], F16, name="maskp")
        nc.sync.dma_start(maskp_sb[:], maskp)
        ind2_sb = const.tile([HEADS, KD], F16, name="ind2")
        nc.sync.dma_start(ind2_sb[:], ind2)
        temp_sb = const.tile([HEADS, 1], F32, name="temp")
        nc.sync.dma_start(temp_sb[:], temp_s)
        outb_sb = const.tile([P, KD], F32, name="outb")
        nc.sync.dma_start(outb_sb[:], outb)

        # --- persistent state ----------------------------------------------
        w_all = persist.tile([P, TOT], F16, name="w_all")
        pi_store = persist.tile([HEADS, n_tokens], F16, name="pi")
        norm2_part = persist.tile([P, KD * NTILES], F32, name="norm2p")
        dots_part = persist.tile([P, KD * NTILES], F32, name="dotsp")
        s_part = persist.tile([HEADS, NG], F32, name="sp")
        inv2 = persist.tile([P, KD], F32, name="inv2")
        lmat = persist.tile([P, KD * HEADS], F16, name="lmat")
        nattn = persist.tile([P, KD], F32, name="nattn")

        # =================== Phase 1: qkv matmul + norm2 + sq spill ========
        with ExitStack() as p1:
            xpool = p1.enter_context(tc.tile_pool(name="x", bufs=3))
            sqpool = p1.enter_context(tc.tile_pool(name="sqst", bufs=2))
            wps = p1.enter_context(tc.tile_pool(name="wps", bufs=6, space="PSUM"))
            for t in range(NTILES):
                xt = xpool.tile([P, TW], F16, tag="x")
                nc.sync.dma_start(xt[:], xb[:, t * TW:(t + 1) * TW])
                sqst = sqpool.tile([P, TW], F16, tag="sqst")
                for kd in range(KD):
                    wp = wps.tile([P, NT], F32, tag="wps")
                    for kc in range(KD):
                        nc.tensor.matmul(
                            wp[:],
                            lhsT=qkvwT_sb[kc][:, kd * P:(kd + 1) * P],
                            rhs=xt[:, kc * NT:(kc + 1) * NT],
                            start=(kc == 0), stop=(kc == KD - 1))
                    wc = w_all[:, t * TW + kd * NT:t * TW + (kd + 1) * NT]
                    nc.vector.tensor_copy(wc, wp[:])
                    acc = norm2_part[:, kd * NTILES + t:kd * NTILES + t + 1]
                    sqc = sqst[:, kd * NT:(kd + 1) * NT]
                    if kd < 2:
                        nc.scalar.activation(sqc, wp[:], AF.Square,
                                             accum_out=acc)
                    else:
                        nc.vector.scalar_tensor_tensor(
                            out=sqc, in0=wc, scalar=1.0, in1=wc,
                            op0=ALU.mult, op1=ALU.mult, accum_out=acc)
                nc.sync.dma_start(sq_dram[:, t * TW:(t + 1) * TW], sqst[:])

            # --- finalize norm2 -> invnorm2*LM_SCALE -> logits lhsT --------
            for kd in range(KD):
                nc.vector.tensor_reduce(
                    inv2[:, kd:kd + 1],
                    norm2_part[:, kd * NTILES:(kd + 1) * NTILES],
                    axis=mybir.AxisListType.X, op=ALU.add)
            nc.vector.reciprocal(inv2[:], inv2[:])
            nc.vector.tensor_scalar_mul(inv2[:], inv2[:], LM_SCALE)
            for kd in range(KD):
                nc.vector.tensor_scalar(
                    lmat[:, kd * HEADS:(kd + 1) * HEADS],
                    lgmask_sb[:, kd * HEADS:(kd + 1) * HEADS],
                    scalar1=inv2[:, kd:kd + 1], scalar2=None, op0=ALU.mult)

        # =================== Phase 2: softmax over heads + dots ============
        with ExitStack() as p2:
            sqin = p2.enter_context(tc.tile_pool(name="sqin", bufs=4))
            pibp = p2.enter_context(tc.tile_pool(name="pib", bufs=2))
            strip = p2.enter_context(tc.tile_pool(name="strip", bufs=2))
            scrp = p2.enter_context(tc.tile_pool(name="scr", bufs=1))
            lgps = p2.enter_context(tc.tile_pool(name="lgps", bufs=2, space="PSUM"))
            smps = p2.enter_context(tc.tile_pool(name="smps", bufs=1, space="PSUM"))
            srbps = p2.enter_context(tc.tile_pool(name="srbps", bufs=1, space="PSUM"))
            for g in range(NG):
                sqts = []
                lg = lgps.tile([HEADS, GW], F32, tag="lg")
                for tt in range(G):
                    t = g * G + tt
                    sqt = sqin.tile([P, TW], F16, tag="sqin")
                    nc.sync.dma_start(sqt[:], sq_dram[:, t * TW:(t + 1) * TW])
                    sqts.append(sqt)
                    for kd in range(KD):
                        nc.tensor.matmul(
                            lg[:, tt * NT:(tt + 1) * NT],
                            lhsT=lmat[:, kd * HEADS:(kd + 1) * HEADS],
                            rhs=sqt[:, kd * NT:(kd + 1) * NT],
                            start=(kd == 0), stop=(kd == KD - 1))
                p16 = strip.tile([HEADS, GW], F16, tag="p16")
                nc.scalar.activation(p16[:], lg[:], AF.Exp,
                                     scale=temp_sb[:, 0:1])
                sm = smps.tile([HEADS, GW], F32, tag="sm")
                for tt in range(G):
                    nc.tensor.matmul(sm[:, tt * NT:(tt + 1) * NT],
                                     lhsT=ones8_sb[:],
                                     rhs=p16[:, tt * NT:(tt + 1) * NT])
                lns = strip.tile([HEADS, GW], F16, tag="lns")
                nc.scalar.activation(lns[:], sm[:], AF.Ln)
                rs = strip.tile([HEADS, GW], F16, tag="rs")
                nc.scalar.activation(rs[:], lns[:], AF.Exp, scale=-1.0)
                pi_g = pi_store[:, g * GW:(g + 1) * GW]
                nc.vector.scalar_tensor_tensor(
                    out=pi_g, in0=p16[:], scalar=1.0, in1=rs[:],
                    op0=ALU.mult, op1=ALU.mult,
                    accum_out=s_part[:, g:g + 1])
                for tt in range(G):
                    t = g * G + tt
                    pib = pibp.tile([P, TW], F16, tag="pib")
                    for kd in range(KD):
                        src = (pi_store[2 * kd:2 * kd + 2,
                                        t * NT:(t + 1) * NT]
                               .rearrange("h (r n) -> h r n", r=1)
                               .broadcast_to([2, HD, NT]))
                        nc.sync.dma_start(
                            pib[:, kd * NT:(kd + 1) * NT], src)
                    scr = scrp.tile([P, NT], F16, tag="scr")
                    for kd in range(KD):
                        nc.vector.scalar_tensor_tensor(
                            out=scr[:],
                            in0=sqts[tt][:, kd * NT:(kd + 1) * NT],
                            scalar=1.0,
                            in1=pib[:, kd * NT:(kd + 1) * NT],
                            op0=ALU.mult, op1=ALU.mult,
                            accum_out=dots_part[:, kd * NTILES + t:
                                                kd * NTILES + t + 1])

            # --- finalize: S, dots, attn, fold -attn into outwT ------------
            svec = strip.tile([HEADS, 1], F32, tag="svec")
            nc.vector.tensor_reduce(svec[:], s_part[:],
                                    axis=mybir.AxisListType.X, op=ALU.add)
            nc.vector.tensor_scalar_add(svec[:], svec[:], 1e-8)
            nc.vector.reciprocal(svec[:], svec[:])
            # permute 1/(S+eps) from head layout to the per-d partition
            # layout with a tiny matmul (PE can cross partitions).
            rsm = strip.tile([HEADS, P], F16, tag="rsm")
            nc.vector.tensor_scalar(
                rsm[:], maskp_sb[:], scalar1=svec[:, 0:1], scalar2=None,
                op0=ALU.mult)
            srb = srbps.tile([P, KD], F32, tag="srb")
            nc.tensor.matmul(srb[:], lhsT=rsm[:], rhs=ind2_sb[:])
            for kd in range(KD):
                nc.vector.tensor_reduce(
                    nattn[:, kd:kd + 1],
                    dots_part[:, kd * NTILES:(kd + 1) * NTILES],
                    axis=mybir.AxisListType.X, op=ALU.add)
            # dots_n = dots/(S+eps); attn = -1/(1+dots_n)
            nc.vector.tensor_tensor(nattn[:], nattn[:], srb[:], op=ALU.mult)
            nc.vector.tensor_scalar_add(nattn[:], nattn[:], 1.0)
            nc.vector.reciprocal(nattn[:], nattn[:])
            nc.vector.tensor_scalar_mul(nattn[:], nattn[:], -1.0)
            for kd in range(KD):
                nc.vector.tensor_scalar(
                    outwT_sb[kd][:], outwT_sb[kd][:],
                    scalar1=nattn[:, kd:kd + 1], scalar2=None, op0=ALU.mult)

        # =================== Phase 3: output + projection ==================
        with ExitStack() as p3:
            pibp3 = p3.enter_context(tc.tile_pool(name="pib3", bufs=2))
            opool = p3.enter_context(tc.tile_pool(name="o", bufs=2))
            ypool = p3.enter_context(tc.tile_pool(name="y", bufs=2))
            yps = p3.enter_context(tc.tile_pool(name="yps", bufs=4, space="PSUM"))
            for t in range(NTILES):
                pib = pibp3.tile([P, TW], F16, tag="pib3")
                for kd in range(KD):
                    src = (pi_store[2 * kd:2 * kd + 2, t * NT:(t + 1) * NT]
                           .rearrange("h (r n) -> h r n", r=1)
                           .broadcast_to([2, HD, NT]))
                    nc.sync.dma_start(pib[:, kd * NT:(kd + 1) * NT], src)
                ot = opool.tile([P, TW], F16, tag="o")
                nc.vector.tensor_mul(
                    ot[:], w_all[:, t * TW:(t + 1) * TW], pib[:])
                yst = ypool.tile([P, TW], F16, tag="y")
                for kc in range(KD):
                    yp = yps.tile([P, NT], F32, tag="yps")
                    for kd in range(KD):
                        nc.tensor.matmul(
                            yp[:],
                            lhsT=outwT_sb[kd][:, kc * P:(kc + 1) * P],
                            rhs=ot[:, kd * NT:(kd + 1) * NT],
                            start=(kd == 0), stop=(kd == KD - 1))
                    yc = yst[:, kc * NT:(kc + 1) * NT]
                    if kc < 2:
                        nc.scalar.activation(yc, yp[:], AF.Identity,
                                             bias=outb_sb[:, kc:kc + 1],
                                             scale=1.0)
                    else:
                        nc.vector.tensor_scalar(
                            yc, yp[:], scalar1=outb_sb[:, kc:kc + 1],
                            scalar2=None, op0=ALU.add)
                nc.sync.dma_start(y[:, t * TW:(t + 1) * TW], yst[:])

    nc.compile()
    _dedupe_act_table_loads(nc)
    return nc


def _host_inputs(x, qkv_w, temp, out_w, out_b):
    NTILES = (x.shape[2] * x.shape[3]) // NT
    qkvwT = np.ascontiguousarray(np.asarray(qkv_w).T).astype(np.float16)
    outwT = np.ascontiguousarray(np.asarray(out_w).T).astype(np.float16)
    # lgmask[p, kd*8+h] = 1 iff h == 2*kd + p//64
    lgmask = np.zeros((P, KD * HEADS), np.float16)
    for p in range(P):
        for kd in range(KD):
            lgmask[p, kd * HEADS + 2 * kd + p // HD] = 1.0
    ones8 = np.ones((HEADS, HEADS), np.float16)
    # maskp[h, p] = 1 iff p//64 == h%2 ; ind2[h, kd] = 1 iff h//2 == kd
    maskp = np.zeros((HEADS, P), np.float16)
    for h in range(HEADS):
        maskp[h, (h % 2) * HD:(h % 2) * HD + HD] = 1.0
    ind2 = np.zeros((HEADS, KD), np.float16)
    for h in range(HEADS):
        ind2[h, h // 2] = 1.0
    temp_sc = (np.asarray(temp, np.float32) / LM_SCALE).reshape(HEADS, 1)
    outb_a = np.asarray(out_b, np.float32).reshape(KD, P).T.copy()
    maps = []
    for i in range(x.shape[0]):
        # t-major layout: xb[p, t*TW + kc*NT + n] = x[kc*128+p, t*NT+n]
        xi = np.asarray(x[i], np.float32).reshape(KD, P, NTILES, NT)
        xi = xi.transpose(1, 2, 0, 3).reshape(P, NTILES * TW)
        maps.append({
            "xb": xi.astype(np.float16),
            "qkvwT": qkvwT, "outwT": outwT, "lgmask": lgmask,
            "ones8": ones8, "maskp": maskp, "ind2": ind2,
            "temp_s": temp_sc, "outb": outb_a,
        })
    return maps


def kernel(x, qkv_w, temp, out_w, out_b):
    x = np.asarray(x)
    b, c, h, w = x.shape
    n_tokens = h * w
    ntiles = n_tokens // NT
    key = (n_tokens, b)
    if key not in _NC_CACHE:
        _NC_CACHE[key] = _build_nc(n_tokens=n_tokens, n_cores=b)
    nc = _NC_CACHE[key]
    in_maps = _host_inputs(x, qkv_w, temp, out_w, out_b)
    res = run_bass_kernel_spmd(nc, in_maps, list(range(b)))
    out = np.empty((b, c, h, w), np.float32)
    for i in range(b):
        yi = res.results[i]["y"].reshape(P, ntiles, KD, NT)
        out[i] = yi.transpose(2, 0, 1, 3).reshape(c, n_tokens) \
            .astype(np.float32).reshape(c, h, w)
    return out


# revision 9
# speedup vs baseline: 2.9702x; 2.9702x over previous
"""AttentionTSSA Trainium2 kernel — full-IO contract.

kernel(**inputs) takes the FULL inputs (x [8,512,128,128], qkv_w, temp,
out_w, out_b), shards data-parallel over batch across the 8 NeuronCores
(batch i -> core i), runs a Bass/Tile kernel per core, and returns the
full [8,512,128,128] float32 output.

Per-core computation, t-major layout: all big operands are
[128 part, 32 tiles * (4 chunks * 512 tokens)] so engine ops and DMAs
are contiguous [128, 512..2048] slices.

  P1 (PE-bound): w = qkv_w @ x (fp16 matmuls); w PSUM->SBUF copies and
     squares (with fused norm2 accumulation) split across ACT/DVE;
     sq spilled to DRAM so phase 2's engines stay free.
  P2: logits replicated across the four 32-partition strips
     (lhsT = inv2-masked replicated indicator) -> [128,1024] strip;
     softmax over heads: exp -> ones-matmul (S replicated to all
     partitions) -> Ln -> exp(-lnS) (division-free reciprocal);
     Pi kept strip-replicated so the head->64-block broadcast runs as
     4 CONCURRENT row-tiled matmuls (tile_position=(32j,0), ~1 MM
     time); dots accumulated by 4 STTs reading the PSUM broadcast.
  P3 (PE-bound): same row-tiled broadcast; o = w * Pi_b (4 STTs);
     y = (out_w * -attn) @ o + b with attn pre-folded into the weights
     (4 per-partition scalar ops); y written as f16, one DMA per tile.

After compile, redundant ACT-table loads are collapsed into a single
natural_log_exp_and_others load (contains exp/ln/square/copy/identity).
"""

import sys

sys.path.insert(0, "/opt/trn_rl_repo")

from contextlib import ExitStack

import numpy as np

import concourse.bass as bass
import concourse.tile as tile
from concourse import bacc, mybir
from concourse.bass_utils import run_bass_kernel_spmd
from concourse.hw_specs import get_activation_tables

F32 = mybir.dt.float32
F16 = mybir.dt.float16   # value paths: x, w, sq, Pi, o, weights, y
AF = mybir.ActivationFunctionType
ALU = mybir.AluOpType

B = 8            # batch == number of cores
C = 512          # channels
H_IMG, W_IMG = 128, 128
N = H_IMG * W_IMG
HEADS = 8
HD = 64          # head dim
NT = 512         # tokens per chunk
KD = 4           # 128-partition chunks of the channel dim
P = 128
TW = KD * NT     # tile width in the t-major layout (2048)
G = 2            # tiles per softmax group (strip FD = G*NT = 1024)
LM_SCALE = 256.0  # keeps invnorm2 out of fp16-subnormal range in lmat

_NC_CACHE = {}


def _dedupe_act_table_loads(nc):
    """Collapse all InstLoadActFuncSet into one load of the set that
    contains every function this kernel uses (exp, ln, square, copy,
    identity). The kernel CFG is a single linear block per engine, so a
    single leading load is sufficient."""
    tables = list(get_activation_tables(nc.m.arch).keys())
    want = {AF.Exp, AF.Ln, AF.Square, AF.Copy, AF.Identity}
    sets = get_activation_tables(nc.m.arch)
    target = None
    for idx, name in enumerate(tables):
        if want <= sets[name]:
            target = idx
            break
    if target is None:
        return
    first = True
    for blk in nc.main_func.blocks:
        keep = []
        for inst in blk.instructions:
            if isinstance(inst, mybir.InstLoadActFuncSet):
                si = inst.sync_info
                has_sync = si is not None and (
                    len(si.on_wait) > 0 or len(si.on_update) > 0)
                if first or has_sync:
                    inst.act_func_set_id = target
                    first = False
                    keep.append(inst)
            else:
                keep.append(inst)
        blk.instructions[:] = keep


def _build_nc(n_tokens=N, n_cores=B):
    NTILES = n_tokens // NT          # 32
    NG = NTILES // G                 # softmax groups
    GW = G * NT                      # strip width (1024)
    TOT = NTILES * TW                # 65536 columns in t-major layout
    nc = bacc.Bacc("TRN2", target_bir_lowering=False, debug=False,
                   num_devices=n_cores)

    xb = nc.dram_tensor("xb", [P, TOT], F16, kind="ExternalInput").ap()
    qkvwT = nc.dram_tensor("qkvwT", [C, C], F16, kind="ExternalInput").ap()
    outwT = nc.dram_tensor("outwT", [C, C], F16, kind="ExternalInput").ap()
    lgmask = nc.dram_tensor("lgmask", [P, KD * P], F16,
                            kind="ExternalInput").ap()
    indrt = nc.dram_tensor("indrt", [P, P], F16, kind="ExternalInput").ap()
    ones8 = nc.dram_tensor("ones8", [HEADS, P], F16,
                           kind="ExternalInput").ap()
    maskp = nc.dram_tensor("maskp", [HEADS, P], F16,
                           kind="ExternalInput").ap()
    ind2 = nc.dram_tensor("ind2", [HEADS, KD], F16,
                          kind="ExternalInput").ap()
    temp_s = nc.dram_tensor("temp_s", [P, 1], F32,
                            kind="ExternalInput").ap()
    outb = nc.dram_tensor("outb", [P, KD], F32, kind="ExternalInput").ap()
    y = nc.dram_tensor("y", [P, TOT], F16, kind="ExternalOutput").ap()
    sq_dram = nc.dram_tensor("sq_scratch", [P, TOT], F16).ap()

    with tile.TileContext(nc) as tc, ExitStack() as top:
        const = top.enter_context(tc.tile_pool(name="const", bufs=1))
        persist = top.enter_context(tc.tile_pool(name="persist", bufs=1))

        # --- constants into SBUF -------------------------------------------
        qkvwT_sb = [const.tile([P, C], F16, name=f"qkvwT{k}") for k in range(KD)]
        outwT_sb = [const.tile([P, C], F16, name=f"outwT{k}") for k in range(KD)]
        for k in range(KD):
            nc.sync.dma_start(qkvwT_sb[k][:], qkvwT[k * P:(k + 1) * P, :])
            nc.sync.dma_start(outwT_sb[k][:], outwT[k * P:(k + 1) * P, :])
        lgmask_sb = const.tile([P, KD * P], F16, name="lgmask")
        nc.sync.dma_start(lgmask_sb[:], lgmask)
        indrt_sb = const.tile([P, P], F16, name="indrt")
        nc.sync.dma_start(indrt_sb[:], indrt)
        ones8_sb = const.tile([HEADS, P], F16, name="ones8")
        nc.sync.dma_start(ones8_sb[:], ones8)
        maskp_sb = const.tile([HEADS, P], F16, name="maskp")
        nc.sync.dma_start(maskp_sb[:], maskp)
        ind2_sb = const.tile([HEADS, KD], F16, name="ind2")
        nc.sync.dma_start(ind2_sb[:], ind2)
        temp_sb = const.tile([P, 1], F32, name="temp")
        nc.sync.dma_start(temp_sb[:], temp_s)
        outb_sb = const.tile([P, KD], F32, name="outb")
        nc.sync.dma_start(outb_sb[:], outb)

        # --- persistent state ----------------------------------------------
        w_all = persist.tile([P, TOT], F16, name="w_all")
        pi_store = persist.tile([P, n_tokens], F16, name="pi")
        norm2_part = persist.tile([P, KD * NTILES], F32, name="norm2p")
        dots_part = persist.tile([P, KD * NTILES], F32, name="dotsp")
        s_part = persist.tile([P, NG], F32, name="sp")
        inv2 = persist.tile([P, KD], F32, name="inv2")
        lmat = persist.tile([P, KD * P], F16, name="lmat")
        nattn = persist.tile([P, KD], F32, name="nattn")

        # =================== Phase 1: qkv matmul + norm2 + sq spill ========
        with ExitStack() as p1:
            xpool = p1.enter_context(tc.tile_pool(name="x", bufs=3))
            sqpool = p1.enter_context(tc.tile_pool(name="sqst", bufs=2))
            wps = p1.enter_context(tc.tile_pool(name="wps", bufs=6, space="PSUM"))
            for t in range(NTILES):
                xt = xpool.tile([P, TW], F16, tag="x")
                nc.sync.dma_start(xt[:], xb[:, t * TW:(t + 1) * TW])
                sqst = sqpool.tile([P, TW], F16, tag="sqst")
                for kd in range(KD):
                    wp = wps.tile([P, NT], F32, tag="wps")
                    for kc in range(KD):
                        nc.tensor.matmul(
                            wp[:],
                            lhsT=qkvwT_sb[kc][:, kd * P:(kd + 1) * P],
                            rhs=xt[:, kc * NT:(kc + 1) * NT],
                            start=(kc == 0), stop=(kc == KD - 1))
                    wc = w_all[:, t * TW + kd * NT:t * TW + (kd + 1) * NT]
                    acc = norm2_part[:, kd * NTILES + t:kd * NTILES + t + 1]
                    sqc = sqst[:, kd * NT:(kd + 1) * NT]
                    if kd < 2:
                        nc.vector.tensor_copy(wc, wp[:])
                        nc.scalar.activation(sqc, wp[:], AF.Square,
                                             accum_out=acc)
                    else:
                        nc.scalar.activation(wc, wp[:], AF.Copy)
                        nc.vector.scalar_tensor_tensor(
                            out=sqc, in0=wc, scalar=1.0, in1=wc,
                            op0=ALU.mult, op1=ALU.mult, accum_out=acc)
                nc.sync.dma_start(sq_dram[:, t * TW:(t + 1) * TW], sqst[:])

            # --- finalize norm2 -> invnorm2*LM_SCALE -> logits lhsT --------
            for kd in range(KD):
                nc.vector.tensor_reduce(
                    inv2[:, kd:kd + 1],
                    norm2_part[:, kd * NTILES:(kd + 1) * NTILES],
                    axis=mybir.AxisListType.X, op=ALU.add)
            nc.vector.reciprocal(inv2[:], inv2[:])
            nc.vector.tensor_scalar_mul(inv2[:], inv2[:], LM_SCALE)
            for kd in range(KD):
                nc.vector.tensor_scalar(
                    lmat[:, kd * P:(kd + 1) * P],
                    lgmask_sb[:, kd * P:(kd + 1) * P],
                    scalar1=inv2[:, kd:kd + 1], scalar2=None, op0=ALU.mult)

        # =================== Phase 2: softmax over heads + dots ============
        with ExitStack() as p2:
            sqin = p2.enter_context(tc.tile_pool(name="sqin", bufs=4))
            strip = p2.enter_context(tc.tile_pool(name="strip", bufs=2))
            scrp = p2.enter_context(tc.tile_pool(name="scr", bufs=1))
            lgps = p2.enter_context(tc.tile_pool(name="lgps", bufs=1, space="PSUM"))
            smps = p2.enter_context(tc.tile_pool(name="smps", bufs=1, space="PSUM"))
            pibs = p2.enter_context(tc.tile_pool(name="pib", bufs=4, space="PSUM"))
            for g in range(NG):
                sqts = []
                lg = lgps.tile([P, GW], F32, tag="lg")
                for tt in range(G):
                    t = g * G + tt
                    sqt = sqin.tile([P, TW], F16, tag="sqin")
                    nc.sync.dma_start(sqt[:], sq_dram[:, t * TW:(t + 1) * TW])
                    sqts.append(sqt)
                    for kd in range(KD):
                        nc.tensor.matmul(
                            lg[:, tt * NT:(tt + 1) * NT],
                            lhsT=lmat[:, kd * P:(kd + 1) * P],
                            rhs=sqt[:, kd * NT:(kd + 1) * NT],
                            start=(kd == 0), stop=(kd == KD - 1))
                # strip-replicated softmax: p16 = exp(temp*logits)
                p16 = strip.tile([P, GW], F16, tag="p16")
                nc.scalar.activation(p16[:], lg[:], AF.Exp,
                                     scale=temp_sb[:, 0:1])
                sm = smps.tile([P, GW], F32, tag="sm")
                for tt in range(G):
                    nc.tensor.matmul(sm[:, tt * NT:(tt + 1) * NT],
                                     lhsT=ones8_sb[:],
                                     rhs=p16[0:HEADS, tt * NT:(tt + 1) * NT])
                lns = strip.tile([P, GW], F16, tag="lns")
                nc.scalar.activation(lns[:], sm[:], AF.Ln)
                rs = strip.tile([P, GW], F16, tag="rs")
                nc.scalar.activation(rs[:], lns[:], AF.Exp, scale=-1.0)
                pi_g = pi_store[:, g * GW:(g + 1) * GW]
                nc.vector.scalar_tensor_tensor(
                    out=pi_g, in0=p16[:], scalar=1.0, in1=rs[:],
                    op0=ALU.mult, op1=ALU.mult,
                    accum_out=s_part[:, g:g + 1])
                for tt in range(G):
                    t = g * G + tt
                    pibt = []
                    for j in range(KD):
                        pib = pibs.tile([P, NT], F32, tag="pib")
                        nc.tensor.matmul(
                            pib[:],
                            lhsT=indrt_sb[32 * j:32 * j + HEADS, :],
                            rhs=pi_store[32 * j:32 * j + HEADS,
                                         t * NT:(t + 1) * NT],
                            tile_position=(32 * j, 0))
                        pibt.append(pib)
                    scr = scrp.tile([P, NT], F16, tag="scr")
                    for kd in range(KD):
                        nc.vector.scalar_tensor_tensor(
                            out=scr[:],
                            in0=sqts[tt][:, kd * NT:(kd + 1) * NT],
                            scalar=1.0,
                            in1=pibt[kd][:],
                            op0=ALU.mult, op1=ALU.mult,
                            accum_out=dots_part[:, kd * NTILES + t:
                                                kd * NTILES + t + 1])

        # =================== Phase 3: attn finalize, output, projection ===
        with ExitStack() as p3:
            fstr = p3.enter_context(tc.tile_pool(name="fstr", bufs=1))
            with tc.tile_pool(name="srbps", bufs=1, space="PSUM") as srbps:
                # S per head, 1/(S+eps), permuted to per-d partition layout
                # with a tiny matmul (PE can cross partitions).
                svec = fstr.tile([HEADS, 1], F32, name="svec")
                nc.vector.tensor_reduce(svec[:], s_part[0:HEADS, :],
                                        axis=mybir.AxisListType.X, op=ALU.add)
                nc.vector.tensor_scalar_add(svec[:], svec[:], 1e-8)
                nc.vector.reciprocal(svec[:], svec[:])
                rsm = fstr.tile([HEADS, P], F16, name="rsm")
                nc.vector.tensor_scalar(
                    rsm[:], maskp_sb[:], scalar1=svec[:, 0:1], scalar2=None,
                    op0=ALU.mult)
                srb = srbps.tile([P, KD], F32, tag="srb")
                nc.tensor.matmul(srb[:], lhsT=rsm[:], rhs=ind2_sb[:])
                for kd in range(KD):
                    nc.vector.tensor_reduce(
                        nattn[:, kd:kd + 1],
                        dots_part[:, kd * NTILES:(kd + 1) * NTILES],
                        axis=mybir.AxisListType.X, op=ALU.add)
                # dots_n = dots/(S+eps); attn = -1/(1+dots_n); fold into W
                nc.vector.tensor_tensor(nattn[:], nattn[:], srb[:],
                                        op=ALU.mult)
                nc.vector.tensor_scalar_add(nattn[:], nattn[:], 1.0)
                nc.vector.reciprocal(nattn[:], nattn[:])
                nc.vector.tensor_scalar_mul(nattn[:], nattn[:], -1.0)
                for kd in range(KD):
                    nc.vector.tensor_scalar(
                        outwT_sb[kd][:], outwT_sb[kd][:],
                        scalar1=nattn[:, kd:kd + 1], scalar2=None,
                        op0=ALU.mult)

            opool = p3.enter_context(tc.tile_pool(name="o", bufs=2))
            ypool = p3.enter_context(tc.tile_pool(name="y", bufs=2))
            pibs3 = p3.enter_context(tc.tile_pool(name="pib3", bufs=4, space="PSUM"))
            yps = p3.enter_context(tc.tile_pool(name="yps", bufs=4, space="PSUM"))
            for t in range(NTILES):
                ot = opool.tile([P, TW], F16, tag="o")
                for j in range(KD):
                    pib = pibs3.tile([P, NT], F32, tag="pib3")
                    nc.tensor.matmul(
                        pib[:],
                        lhsT=indrt_sb[32 * j:32 * j + HEADS, :],
                        rhs=pi_store[32 * j:32 * j + HEADS,
                                     t * NT:(t + 1) * NT],
                        tile_position=(32 * j, 0))
                    nc.vector.scalar_tensor_tensor(
                        out=ot[:, j * NT:(j + 1) * NT],
                        in0=w_all[:, t * TW + j * NT:t * TW + (j + 1) * NT],
                        scalar=1.0, in1=pib[:],
                        op0=ALU.mult, op1=ALU.mult)
                yst = ypool.tile([P, TW], F16, tag="y")
                for kc in range(KD):
                    yp = yps.tile([P, NT], F32, tag="yps")
                    for kd in range(KD):
                        nc.tensor.matmul(
                            yp[:],
                            lhsT=outwT_sb[kd][:, kc * P:(kc + 1) * P],
                            rhs=ot[:, kd * NT:(kd + 1) * NT],
                            start=(kd == 0), stop=(kd == KD - 1))
                    nc.scalar.activation(yst[:, kc * NT:(kc + 1) * NT],
                                         yp[:], AF.Identity,
                                         bias=outb_sb[:, kc:kc + 1],
                                         scale=1.0)
                nc.sync.dma_start(y[:, t * TW:(t + 1) * TW], yst[:])

    nc.compile()
    _dedupe_act_table_loads(nc)
    return nc


def _host_inputs(x, qkv_w, temp, out_w, out_b):
    NTILES = (x.shape[2] * x.shape[3]) // NT
    qkvwT = np.ascontiguousarray(np.asarray(qkv_w).T).astype(np.float16)
    outwT = np.ascontiguousarray(np.asarray(out_w).T).astype(np.float16)
    # lgmask[p, kd*128 + 32j+h] = 1 iff h == 2*kd + p//64  (strip-replicated)
    lgmask = np.zeros((P, KD * P), np.float16)
    for p in range(P):
        for kd in range(KD):
            for j in range(KD):
                lgmask[p, kd * P + 32 * j + 2 * kd + p // HD] = 1.0
    # indrt[32j+h, p] = 1 iff h == 2j + p//64 (row-tiled broadcast lhsT)
    indrt = np.zeros((P, P), np.float16)
    for j in range(KD):
        for p in range(P):
            indrt[32 * j + 2 * j + p // HD, p] = 1.0
    ones8 = np.ones((HEADS, P), np.float16)
    # maskp[h, p] = 1 iff p//64 == h%2 ; ind2[h, kd] = 1 iff h//2 == kd
    maskp = np.zeros((HEADS, P), np.float16)
    for h in range(HEADS):
        maskp[h, (h % 2) * HD:(h % 2) * HD + HD] = 1.0
    ind2 = np.zeros((HEADS, KD), np.float16)
    for h in range(HEADS):
        ind2[h, h // 2] = 1.0
    # temp replicated to the strip layout: temp_rep[32j+h] = temp[h]/LM
    tarr = np.asarray(temp, np.float32).reshape(HEADS)
    temp_rep = np.zeros((P, 1), np.float32)
    for j in range(KD):
        temp_rep[32 * j:32 * j + HEADS, 0] = tarr / LM_SCALE
    outb_a = np.asarray(out_b, np.float32).reshape(KD, P).T.copy()
    maps = []
    for i in range(x.shape[0]):
        # t-major layout: xb[p, t*TW + kc*NT + n] = x[kc*128+p, t*NT+n]
        xi = np.asarray(x[i], np.float32).reshape(KD, P, NTILES, NT)
        xi = xi.transpose(1, 2, 0, 3).reshape(P, NTILES * TW)
        maps.append({
            "xb": xi.astype(np.float16),
            "qkvwT": qkvwT, "outwT": outwT, "lgmask": lgmask,
            "indrt": indrt, "ones8": ones8, "maskp": maskp, "ind2": ind2,
            "temp_s": temp_rep, "outb": outb_a,
        })
    return maps


def kernel(x, qkv_w, temp, out_w, out_b):
    x = np.asarray(x)
    b, c, h, w = x.shape
    n_tokens = h * w
    ntiles = n_tokens // NT
    key = (n_tokens, b)
    if key not in _NC_CACHE:
        _NC_CACHE[key] = _build_nc(n_tokens=n_tokens, n_cores=b)
    nc = _NC_CACHE[key]
    in_maps = _host_inputs(x, qkv_w, temp, out_w, out_b)
    res = run_bass_kernel_spmd(nc, in_maps, list(range(b)))
    out = np.empty((b, c, h, w), np.float32)
    for i in range(b):
        yi = res.results[i]["y"].reshape(P, ntiles, KD, NT)
        out[i] = yi.transpose(2, 0, 1, 3).reshape(c, n_tokens) \
            .astype(np.float32).reshape(c, h, w)
    return out
